# revision 10
# baseline (speedup 1.0000x reference)
"""BiLSTM-CRF Trainium kernel — full network on device (raw Bass).

Device (each of 8 cores runs the identical full-batch program; the LSTM
recurrence is latency-bound, not work-bound, so batch-sharding buys
nothing — core 0's output is used):
  - x_proj fused into the recurrence: per step, gates accumulate
    emb@Wih + bias + h@Whh directly in PSUM (no xp HBM round trip).
  - Gate layout: PSUM [128, 512]; partition p = d*64 + hf*32 + b
    (d=direction, hf=half of H, b=batch), free = [i|f|o|g] x 128
    half-columns.  Four 32-partition column groups -> 4-way concurrent
    matmuls via explicit tile_position.
  - sigmoid/tanh on ScalarE, cell update on VectorE, h transposed back to
    stationary layout ([h-col, (d,hf,b)]) with a PE transpose each step.
  - Final feats GEMM (hs @ Wout^T) on device -> featsT per direction.
Raw Bass with manual semaphores: this walrus build rejects instructions
carrying more than one sync wait, so every wait is a standalone
sequencer instruction and every instruction carries at most one inc.
Host: embedding gather, Viterbi max/argmax + backtrace (pure int logic).
"""

import os
import numpy as np

V, E, HD, B, S, T = 50000, 256, 512, 32, 256, 24
H = HD // 2
NCORES = 8
SBW = S * B  # 8192 (s, b) columns

LAST_EXEC_NS = None
LAST_RESULT = None


def _build_bass():
    import concourse.bass as bass
    import concourse.mybir as mybir
    from concourse import masks
    from contextlib import ExitStack

    f32, bf16 = mybir.dt.float32, mybir.dt.bfloat16
    AF = mybir.ActivationFunctionType

    nc = bass.Bass(disable_frame_to_traceback=True)
    embT = nc.dram_tensor("embT", [E, SBW], bf16, kind="ExternalInput")
    wrec = nc.dram_tensor("wrec", [E, 2048], bf16, kind="ExternalInput")
    wx = nc.dram_tensor("wx", [E, 2048], bf16, kind="ExternalInput")
    biasd = nc.dram_tensor("biasd", [1, 2048], bf16, kind="ExternalInput")
    woutd = nc.dram_tensor("woutd", [128, 96], bf16, kind="ExternalInput")
    featsT = nc.dram_tensor("featsT", [T, 2 * SBW], f32, kind="ExternalOutput")

    ctx = ExitStack()
    _n = [0]

    def sbt(shape, dt, nm=None):
        _n[0] += 1
        return ctx.enter_context(nc.sbuf_tensor(nm or f"sb{_n[0]}", shape, dt))

    def pst(shape, nm=None, dt=None):
        _n[0] += 1
        return ctx.enter_context(nc.psum_tensor(nm or f"ps{_n[0]}", shape, dt or f32))

    def sem(nm=None):
        _n[0] += 1
        return ctx.enter_context(nc.semaphore(name=nm or f"sem{_n[0]}"))

    with ctx:
        emb_sb = [sbt([128, SBW], bf16) for _ in range(2)]
        wrec_sb = [sbt([128, 2048], bf16) for _ in range(2)]
        wx_sb = [sbt([128, 2048], bf16) for _ in range(2)]
        bias_sb = sbt([1, 2048], bf16)
        wout_sb = sbt([128, 96], bf16)
        ones_sb = sbt([1, 32], bf16)
        ident = sbt([128, 128], bf16)
        hT_all = sbt([128, (S + 1) * 128], bf16)
        sg = [sbt([128, 512], bf16) for _ in range(2)]
        cbuf = [sbt([128, 128], bf16) for _ in range(2)]
        p1 = sbt([128, 128], bf16)
        p2 = sbt([128, 128], bf16)
        gbuf = sbt([128, 128], bf16)
        thb = [sbt([128, 128], bf16) for _ in range(2)]
        hb = [sbt([128, 128], bf16) for _ in range(2)]
        febuf = [sbt([128, 512], f32) for _ in range(2)]
        gb = [pst([128, 512]) for _ in range(4)]
        htp = [pst([128, 128], dt=bf16) for _ in range(2)]
        heat = pst([128, 512])

        load = sem()
        ready = sem()
        gates = sem()
        gread = sem()
        sig = sem()
        csem = sem()
        thsem = sem()
        hraw = sem()
        tp = sem()
        hTs = sem()
        dved = sem()
        fmm = sem()
        fev = sem()
        fout = sem()

        MMKW = dict(skip_group_check=True)

        with nc.Block() as block:

            @block.gpsimd
            def _(g):
                n = 0
                for k in range(2):
                    g.dma_start(emb_sb[k][:], embT[128 * k : 128 * (k + 1), :]).then_inc(load, 16)
                    g.dma_start(wrec_sb[k][:], wrec[128 * k : 128 * (k + 1), :]).then_inc(load, 16)
                    g.dma_start(wx_sb[k][:], wx[128 * k : 128 * (k + 1), :]).then_inc(load, 16)
                    n += 48
                g.dma_start(bias_sb[:], biasd[:]).then_inc(load, 16)
                g.dma_start(wout_sb[:], woutd[:]).then_inc(load, 16)
                n += 32
                g.memset(ones_sb[:], 1.0)
                masks.make_identity(nc, ident[:])
                g.memset(hT_all[:, 0:128], 0.0)
                g.wait_ge(load, n)
                g.nop().then_inc(ready, 1)
                for i in range(32):
                    d, ncnk = divmod(i, 16)
                    g.wait_ge(fev, i + 1)
                    g.dma_start(
                        featsT[:, SBW * d + 512 * ncnk : SBW * d + 512 * (ncnk + 1)],
                        febuf[i % 2][0:T, :],
                    ).then_inc(fout, 16)

            @block.tensor
            def _(t):
                t.wait_ge(ready, 1)
                for s in range(S):
                    bank = gb[s % 4]
                    if s >= 4:
                        t.wait_ge(gread, s - 3)
                    for k in range(2):  # x-part (h-independent)
                        for j in range(4):
                            d = j // 2
                            cs = s if d == 0 else S - 1 - s
                            t.matmul(
                                bank[32 * j : 32 * (j + 1), :],
                                emb_sb[k][:, 32 * cs : 32 * (cs + 1)],
                                wx_sb[k][:, 512 * j : 512 * (j + 1)],
                                start=(k == 0), stop=False,
                                tile_position=(0, 32 * j), **MMKW,
                            )
                    for j in range(4):  # bias
                        t.matmul(
                            bank[32 * j : 32 * (j + 1), :],
                            ones_sb[0:1, 0:32],
                            bias_sb[0:1, 512 * j : 512 * (j + 1)],
                            start=False, stop=False,
                            tile_position=(0, 32 * j), **MMKW,
                        )
                    for _hx in range(6):  # PE heaters: keep HAM at 2.4 GHz
                        t.matmul(
                            heat[0:32, :], emb_sb[0][:, 0:32], wx_sb[0][:, 0:512],
                            start=True, stop=True, tile_position=(0, 0), **MMKW,
                        )
                    if s >= 1:  # transpose previous step's h
                        t.wait_ge(hraw, s)
                        t.transpose(htp[(s - 1) % 2][:], hb[(s - 1) % 2][:], ident[:]).then_inc(tp, 1)
                    for _hx in range(2):  # bridge the evac wait
                        t.matmul(
                            heat[0:32, :], emb_sb[0][:, 0:32], wx_sb[0][:, 0:512],
                            start=True, stop=True, tile_position=(0, 0), **MMKW,
                        )
                    t.wait_ge(hTs, s)
                    mm = None
                    for k in range(2):  # h-part (the recurrence)
                        for j in range(4):
                            d = j // 2
                            col = 128 * s + 64 * d + 32 * k
                            mm = t.matmul(
                                bank[32 * j : 32 * (j + 1), :],
                                hT_all[:, col : col + 32],
                                wrec_sb[k][:, 512 * j : 512 * (j + 1)],
                                start=False, stop=(k == 1),
                                tile_position=(0, 32 * j), **MMKW,
                            )
                    mm.then_inc(gates, 1)
                t.wait_ge(hraw, S)
                t.transpose(htp[(S - 1) % 2][:], hb[(S - 1) % 2][:], ident[:]).then_inc(tp, 1)
                t.wait_ge(gread, S)
                t.wait_ge(hTs, S)
                hT3 = hT_all[:].rearrange("p (t x) -> p t x", x=128)
                for i in range(32):  # feats GEMM
                    d, ncnk = divmod(i, 16)
                    if i >= 2:
                        t.wait_ge(fev, i - 1)
                    mm = None
                    for hf in range(2):
                        j = d * 2 + hf
                        mm = t.matmul(
                            gb[i % 2][0:T, :],
                            wout_sb[:, 24 * j : 24 * (j + 1)],
                            hT3[:, 1 + 16 * ncnk : 1 + 16 * (ncnk + 1), 64 * d + 32 * hf : 64 * d + 32 * hf + 32],
                            start=(hf == 0), stop=(hf == 1), **MMKW,
                        )
                    mm.then_inc(fmm, 1)

            @block.scalar
            def _(a):
                a.wait_ge(ready, 1)
                for s in range(S):
                    bank = gb[s % 4]
                    if s >= 2:
                        a.wait_ge(dved, s - 1)  # sg[s%2] consumed by step s-2
                    a.wait_ge(gates, s + 1)
                    a.activation(sg[s % 2][:], bank[:], AF.Sigmoid).then_inc(sig, 1)
                    a.nop().then_inc(gread, 1)  # bank s%4 free for PE step s+4
                    if s >= 2:
                        a.wait_ge(hraw, s - 1)  # thb[s%2] last read by h-mul(s-2)
                    a.wait_ge(csem, s + 1)
                    a.activation(thb[s % 2][:], cbuf[s % 2][:], AF.Tanh).then_inc(thsem, 1)

            @block.vector
            def _(v):
                v.wait_ge(ready, 1)
                v.memset(cbuf[1][:], 0.0)
                for s in range(S):
                    v.wait_ge(sig, s + 1)
                    v.tensor_mul(p2[:], sg[s % 2][:, 128:256], cbuf[(s + 1) % 2][:])
                    v.tensor_scalar(gbuf[:], sg[s % 2][:, 384:512], 2.0, -1.0,
                                    op0=mybir.AluOpType.mult, op1=mybir.AluOpType.add)
                    v.tensor_mul(p1[:], sg[s % 2][:, 0:128], gbuf[:])
                    v.tensor_add(cbuf[s % 2][:], p1[:], p2[:]).then_inc(csem, 1)
                    v.wait_ge(thsem, s + 1)
                    v.tensor_mul(hb[s % 2][:], sg[s % 2][:, 256:384], thb[s % 2][:]).then_inc(hraw, 1)
                    v.engine_nop().then_inc(dved, 1)
                    v.wait_ge(tp, s + 1)
                    v.tensor_copy(hT_all[:, 128 * (s + 1) : 128 * (s + 2)], htp[s % 2][:]).then_inc(hTs, 1)
                for i in range(32):
                    v.wait_ge(fmm, i + 1)
                    if i >= 2:
                        v.wait_ge(fout, 16 * (i - 1))  # out-DMA done before reuse
                    v.tensor_copy(febuf[i % 2][0:T, :], gb[i % 2][0:T, :]).then_inc(fev, 1)

    return nc


_GORDER = (0, 1, 3, 2)  # free-layout gate slots [i|f|o|g] -> pytorch row blocks


def _prep_w(Whh, n_in):
    out = []
    for hf in range(2):
        cols = []
        for gi in _GORDER:
            blk = Whh[256 * gi + 128 * hf : 256 * gi + 128 * hf + 128, :]  # [128, n_in]
            cols.append(blk.T * (2.0 if gi == 2 else 1.0))
        out.append(np.concatenate(cols, axis=1))  # [n_in, 512]
    return out


def _prep_b(bsum):
    out = []
    for hf in range(2):
        cols = [bsum[256 * gi + 128 * hf : 256 * gi + 128 * hf + 128] * (2.0 if gi == 2 else 1.0) for gi in _GORDER]
        out.append(np.concatenate(cols))
    return out


def _install_trace_shim():
    # antenv.axon_hooks is absent in this image; synthesize it from the
    # ctypes hook that ships in trn_agent_boot so trace=True works.
    import sys, types
    try:
        import antenv.axon_hooks  # noqa: F401
        return
    except ImportError:
        pass
    from trn_agent_boot.trn_boot import _ntff_profile_via_ctypes
    hook = _ntff_profile_via_ctypes('/opt/axon/libaxon_pjrt.so')
    mod = types.ModuleType('antenv.axon_hooks')
    mod.get_axon_ntff_profile_hook = lambda: hook
    mod.set_axon_ntff_profile_hook = lambda h: None
    sys.modules['antenv.axon_hooks'] = mod
    import concourse.bass_utils as bu
    bu.upload_artifacts = lambda tmpdir: ""  # zero-egress container


def _run_device(embT_np, wrec_np, wx_np, bias_np, wout_np, trace=False):
    global LAST_EXEC_NS
    from concourse.bass_utils import run_bass_kernel_spmd

    if trace:
        _install_trace_shim()

    nc = _build_bass()
    in_map = {
        "embT": embT_np,
        "wrec": wrec_np,
        "wx": wx_np,
        "biasd": bias_np,
        "woutd": wout_np,
    }
    res = run_bass_kernel_spmd(
        nc, [in_map] * NCORES, core_ids=list(range(NCORES)), trace=trace
    )
    LAST_EXEC_NS = res.exec_time_ns
    global LAST_RESULT
    LAST_RESULT = res
    return res.results[0]["featsT"]


def _host_feats(emb, Wih_f, Whh_f, bih_f, bhh_f, Wih_b, Whh_b, bih_b, bhh_b, Wout, bout):
    xs = np.swapaxes(emb, 0, 1).astype(np.float32)  # [S, B, E]

    def sigmoid(x):
        return 1.0 / (1.0 + np.exp(-x))

    def run_dir(Wih, Whh, bih, bhh, reverse):
        xp = xs @ Wih.T.astype(np.float32) + (bih + bhh).astype(np.float32)
        WhhT = np.ascontiguousarray(Whh.T.astype(np.float32))
        h = np.zeros((B, H), np.float32)
        c = np.zeros((B, H), np.float32)
        hs = np.empty((S, B, H), np.float32)
        order = range(S - 1, -1, -1) if reverse else range(S)
        for s in order:
            gg = xp[s] + h @ WhhT
            i = sigmoid(gg[:, :H])
            f = sigmoid(gg[:, H : 2 * H])
            gq = np.tanh(gg[:, 2 * H : 3 * H])
            o = sigmoid(gg[:, 3 * H :])
            c = f * c + i * gq
            h = o * np.tanh(c)
            hs[s] = h
        return hs

    hf = run_dir(Wih_f, Whh_f, bih_f, bhh_f, False)
    hb = run_dir(Wih_b, Whh_b, bih_b, bhh_b, True)
    hs = np.concatenate([hf, hb], axis=-1)
    return hs @ Wout.T.astype(np.float32) + bout.astype(np.float32)


def kernel(
    sentence,
    embed,
    Wih_f,
    Whh_f,
    bih_f,
    bhh_f,
    Wih_b,
    Whh_b,
    bih_b,
    bhh_b,
    Wout,
    bout,
    transitions,
    start_t,
    stop_t,
):
    import ml_dtypes

    bf16 = ml_dtypes.bfloat16
    sentence = np.asarray(sentence)

    emb = embed.astype(np.float32)[sentence]  # [B, S, E]
    embT_np = np.ascontiguousarray(emb.transpose(2, 1, 0).reshape(E, SBW)).astype(bf16)

    wrec_np = np.concatenate(
        _prep_w(Whh_f.astype(np.float32), H) + _prep_w(Whh_b.astype(np.float32), H),
        axis=1,
    ).astype(bf16)  # [256, 2048]
    wx_np = np.concatenate(
        _prep_w(Wih_f.astype(np.float32), E) + _prep_w(Wih_b.astype(np.float32), E),
        axis=1,
    ).astype(bf16)
    bias_np = np.concatenate(
        _prep_b((bih_f + bhh_f).astype(np.float32))
        + _prep_b((bih_b + bhh_b).astype(np.float32))
    ).reshape(1, 2048).astype(bf16)
    wo = Wout.astype(np.float32)  # [T, 512]
    wout_np = np.concatenate(
        [wo[:, 256 * d + 128 * hf : 256 * d + 128 * hf + 128].T for d in range(2) for hf in range(2)],
        axis=1,
    ).astype(bf16)  # [128, 96]

    trace = bool(int(os.environ.get("KERNEL_TRACE", "0")))
    try:
        featsT = _run_device(embT_np, wrec_np, wx_np, bias_np, wout_np, trace=trace)
        ff = np.asarray(featsT[:, :SBW], dtype=np.float32).reshape(T, S, B)
        fb = np.asarray(featsT[:, SBW:], dtype=np.float32).reshape(T, S, B)[:, ::-1, :]
        feats = (ff + fb).transpose(1, 2, 0)  # [S, B, T]
        feats = feats + bout.astype(np.float32)
    except Exception:
        import traceback

        traceback.print_exc()
        feats = _host_feats(
            emb, Wih_f, Whh_f, bih_f, bhh_f, Wih_b, Whh_b, bih_b, bhh_b, Wout, bout
        )

    # ---- Viterbi (host) ----
    trans = transitions.astype(np.float32)
    v = feats[0] + start_t.astype(np.float32)[None, :]  # [B, T]
    idxs = np.empty((S - 1, B, T), dtype=np.int32)
    for s in range(1, S):
        scores = v[:, :, None] + trans[None]  # [B, prevT, nextT]
        idxs[s - 1] = np.argmax(scores, axis=1)
        v = np.max(scores, axis=1) + feats[s]
    last = np.argmax(v + stop_t.astype(np.float32)[None, :], axis=1).astype(np.int32)

    tags = np.empty((S, B), dtype=np.int32)
    tags[S - 1] = last
    cur = last
    ar = np.arange(B)
    for s in range(S - 2, -1, -1):
        cur = idxs[s][ar, cur].astype(np.int32)
        tags[s] = cur
    return np.ascontiguousarray(tags.T).astype(np.int32)  # [B, S]


# revision 11
# speedup vs baseline: 1.0032x; 1.0032x over previous
"""BiLSTM-CRF Trainium kernel — full network on device (raw Bass).

Device (each of 8 cores runs the identical full-batch program; the LSTM
recurrence is latency-bound, not work-bound, so batch-sharding buys
nothing — core 0's output is used):
  - x_proj fused into the recurrence: per step, gates accumulate
    emb@Wih + bias + h@Whh directly in PSUM (no xp HBM round trip).
  - Gate layout: PSUM [128, 512]; partition p = d*64 + hf*32 + b
    (d=direction, hf=half of H, b=batch), free = [i|f|o|g] x 128
    half-columns.  Four 32-partition column groups -> 4-way concurrent
    matmuls via explicit tile_position.
  - sigmoid/tanh on ScalarE, cell update on VectorE, h transposed back to
    stationary layout ([h-col, (d,hf,b)]) with a PE transpose each step.
  - Final feats GEMM (hs @ Wout^T) on device -> featsT per direction.
Raw Bass with manual semaphores: this walrus build rejects instructions
carrying more than one sync wait, so every wait is a standalone
sequencer instruction and every instruction carries at most one inc.
Host: embedding gather, Viterbi max/argmax + backtrace (pure int logic).
"""

import os
import numpy as np

V, E, HD, B, S, T = 50000, 256, 512, 32, 256, 24
H = HD // 2
NCORES = 8
SBW = S * B  # 8192 (s, b) columns

LAST_EXEC_NS = None
LAST_RESULT = None


def _build_bass():
    import concourse.bass as bass
    import concourse.mybir as mybir
    from concourse import masks
    from contextlib import ExitStack

    f32, bf16 = mybir.dt.float32, mybir.dt.bfloat16
    AF = mybir.ActivationFunctionType

    nc = bass.Bass(disable_frame_to_traceback=True)
    embT = nc.dram_tensor("embT", [E, SBW], bf16, kind="ExternalInput")
    wrec = nc.dram_tensor("wrec", [E, 2048], bf16, kind="ExternalInput")
    wx = nc.dram_tensor("wx", [E, 2048], bf16, kind="ExternalInput")
    biasd = nc.dram_tensor("biasd", [1, 2048], bf16, kind="ExternalInput")
    woutd = nc.dram_tensor("woutd", [128, 96], bf16, kind="ExternalInput")
    featsT = nc.dram_tensor("featsT", [T, 2 * SBW], f32, kind="ExternalOutput")

    ctx = ExitStack()
    _n = [0]

    def sbt(shape, dt, nm=None):
        _n[0] += 1
        return ctx.enter_context(nc.sbuf_tensor(nm or f"sb{_n[0]}", shape, dt))

    def pst(shape, nm=None, dt=None):
        _n[0] += 1
        return ctx.enter_context(nc.psum_tensor(nm or f"ps{_n[0]}", shape, dt or f32))

    def sem(nm=None):
        _n[0] += 1
        return ctx.enter_context(nc.semaphore(name=nm or f"sem{_n[0]}"))

    with ctx:
        emb_sb = [sbt([128, SBW], bf16) for _ in range(2)]
        wrec_sb = [sbt([128, 2048], bf16) for _ in range(2)]
        wx_sb = [sbt([128, 2048], bf16) for _ in range(2)]
        bias_sb = sbt([1, 2048], bf16)
        wout_sb = sbt([128, 96], bf16)
        ones_sb = sbt([1, 32], bf16)
        ident = sbt([128, 128], bf16)
        hT_all = sbt([128, (S + 1) * 128], bf16)
        sg = [sbt([128, 512], bf16) for _ in range(2)]
        cbuf = [sbt([128, 128], bf16) for _ in range(2)]
        p1 = sbt([128, 128], bf16)
        p2 = sbt([128, 128], bf16)
        gbuf = sbt([128, 128], bf16)
        thb = [sbt([128, 128], bf16) for _ in range(2)]
        hb = [sbt([128, 128], bf16) for _ in range(2)]
        febuf = [sbt([128, 512], f32) for _ in range(2)]
        gb = [pst([128, 512]) for _ in range(4)]
        htp = [pst([128, 128], dt=bf16) for _ in range(2)]
        heat = pst([128, 512])

        load = sem()
        ready = sem()
        ready2 = sem()
        gates = sem()
        gread = sem()
        sig = sem()
        csem = sem()
        thsem = sem()
        hraw = sem()
        tp = sem()
        hTs = sem()
        dved = sem()
        fmm = sem()
        fev = sem()
        fout = sem()

        MMKW = dict(skip_group_check=True)

        with nc.Block() as block:

            @block.gpsimd
            def _(g):
                for k in range(2):
                    g.dma_start(emb_sb[k][:], embT[128 * k : 128 * (k + 1), :]).then_inc(load, 16)
                for k in range(2):
                    g.dma_start(wx_sb[k][:], wx[128 * k : 128 * (k + 1), :]).then_inc(load, 16)
                g.dma_start(bias_sb[:], biasd[:]).then_inc(load, 16)
                for k in range(2):
                    g.dma_start(wrec_sb[k][:], wrec[128 * k : 128 * (k + 1), :]).then_inc(load, 16)
                g.dma_start(wout_sb[:], woutd[:]).then_inc(load, 16)
                g.memset(ones_sb[:], 1.0)
                masks.make_identity(nc, ident[:])
                g.memset(hT_all[:, 0:128], 0.0)
                g.wait_ge(load, 80)
                g.nop().then_inc(ready, 1)
                g.wait_ge(load, 128)
                g.nop().then_inc(ready2, 1)
                for i in range(32):
                    d, ncnk = divmod(i, 16)
                    g.wait_ge(fev, i + 1)
                    g.dma_start(
                        featsT[:, SBW * d + 512 * ncnk : SBW * d + 512 * (ncnk + 1)],
                        febuf[i % 2][0:T, :],
                    ).then_inc(fout, 16)

            @block.tensor
            def _(t):
                t.wait_ge(ready, 1)
                for s in range(S):
                    bank = gb[s % 4]
                    if s >= 4:
                        t.wait_ge(gread, s - 3)
                    for k in range(2):  # x-part (h-independent)
                        for j in range(4):
                            d = j // 2
                            cs = s if d == 0 else S - 1 - s
                            t.matmul(
                                bank[32 * j : 32 * (j + 1), :],
                                emb_sb[k][:, 32 * cs : 32 * (cs + 1)],
                                wx_sb[k][:, 512 * j : 512 * (j + 1)],
                                start=(k == 0), stop=False,
                                tile_position=(0, 32 * j), **MMKW,
                            )
                    for j in range(4):  # bias
                        t.matmul(
                            bank[32 * j : 32 * (j + 1), :],
                            ones_sb[0:1, 0:32],
                            bias_sb[0:1, 512 * j : 512 * (j + 1)],
                            start=False, stop=False,
                            tile_position=(0, 32 * j), **MMKW,
                        )
                    for _hx in range(6):  # PE heaters: keep HAM at 2.4 GHz
                        t.matmul(
                            heat[0:32, :], emb_sb[0][:, 0:32], wx_sb[0][:, 0:512],
                            start=True, stop=True, tile_position=(0, 0), **MMKW,
                        )
                    if s >= 1:  # transpose previous step's h
                        t.wait_ge(hraw, s)
                        t.transpose(htp[(s - 1) % 2][:], hb[(s - 1) % 2][:], ident[:]).then_inc(tp, 1)
                    for _hx in range(2):  # bridge the evac wait
                        t.matmul(
                            heat[0:32, :], emb_sb[0][:, 0:32], wx_sb[0][:, 0:512],
                            start=True, stop=True, tile_position=(0, 0), **MMKW,
                        )
                    if s == 0:
                        t.wait_ge(ready2, 1)
                    t.wait_ge(hTs, s)
                    mm = None
                    for k in range(2):  # h-part (the recurrence)
                        for j in range(4):
                            d = j // 2
                            col = 128 * s + 64 * d + 32 * k
                            mm = t.matmul(
                                bank[32 * j : 32 * (j + 1), :],
                                hT_all[:, col : col + 32],
                                wrec_sb[k][:, 512 * j : 512 * (j + 1)],
                                start=False, stop=(k == 1),
                                tile_position=(0, 32 * j), **MMKW,
                            )
                    mm.then_inc(gates, 1)
                t.wait_ge(hraw, S)
                t.transpose(htp[(S - 1) % 2][:], hb[(S - 1) % 2][:], ident[:]).then_inc(tp, 1)
                t.wait_ge(gread, S)
                t.wait_ge(hTs, S)
                hT3 = hT_all[:].rearrange("p (t x) -> p t x", x=128)
                for i in range(32):  # feats GEMM
                    d, ncnk = divmod(i, 16)
                    if i >= 2:
                        t.wait_ge(fev, i - 1)
                    mm = None
                    for hf in range(2):
                        j = d * 2 + hf
                        mm = t.matmul(
                            gb[i % 2][0:T, :],
                            wout_sb[:, 24 * j : 24 * (j + 1)],
                            hT3[:, 1 + 16 * ncnk : 1 + 16 * (ncnk + 1), 64 * d + 32 * hf : 64 * d + 32 * hf + 32],
                            start=(hf == 0), stop=(hf == 1), **MMKW,
                        )
                    mm.then_inc(fmm, 1)

            @block.scalar
            def _(a):
                a.wait_ge(ready2, 1)
                for s in range(S):
                    bank = gb[s % 4]
                    if s >= 2:
                        a.wait_ge(dved, s - 1)  # sg[s%2] consumed by step s-2
                    a.wait_ge(gates, s + 1)
                    a.activation(sg[s % 2][:], bank[:], AF.Sigmoid).then_inc(sig, 1)
                    a.nop().then_inc(gread, 1)  # bank s%4 free for PE step s+4
                    if s >= 2:
                        a.wait_ge(hraw, s - 1)  # thb[s%2] last read by h-mul(s-2)
                    a.wait_ge(csem, s + 1)
                    a.activation(thb[s % 2][:], cbuf[s % 2][:], AF.Tanh).then_inc(thsem, 1)

            @block.vector
            def _(v):
                v.wait_ge(ready2, 1)
                v.memset(cbuf[1][:], 0.0)
                for s in range(S):
                    v.wait_ge(sig, s + 1)
                    v.tensor_mul(p2[:], sg[s % 2][:, 128:256], cbuf[(s + 1) % 2][:])
                    v.tensor_scalar(gbuf[:], sg[s % 2][:, 384:512], 2.0, -1.0,
                                    op0=mybir.AluOpType.mult, op1=mybir.AluOpType.add)
                    v.tensor_mul(p1[:], sg[s % 2][:, 0:128], gbuf[:])
                    v.tensor_add(cbuf[s % 2][:], p1[:], p2[:]).then_inc(csem, 1)
                    v.wait_ge(thsem, s + 1)
                    v.tensor_mul(hb[s % 2][:], sg[s % 2][:, 256:384], thb[s % 2][:]).then_inc(hraw, 1)
                    v.engine_nop().then_inc(dved, 1)
                    v.wait_ge(tp, s + 1)
                    v.tensor_copy(hT_all[:, 128 * (s + 1) : 128 * (s + 2)], htp[s % 2][:]).then_inc(hTs, 1)
                for i in range(32):
                    v.wait_ge(fmm, i + 1)
                    if i >= 2:
                        v.wait_ge(fout, 16 * (i - 1))  # out-DMA done before reuse
                    v.tensor_copy(febuf[i % 2][0:T, :], gb[i % 2][0:T, :]).then_inc(fev, 1)

    return nc


_GORDER = (0, 1, 3, 2)  # free-layout gate slots [i|f|o|g] -> pytorch row blocks


def _prep_w(Whh, n_in):
    out = []
    for hf in range(2):
        cols = []
        for gi in _GORDER:
            blk = Whh[256 * gi + 128 * hf : 256 * gi + 128 * hf + 128, :]  # [128, n_in]
            cols.append(blk.T * (2.0 if gi == 2 else 1.0))
        out.append(np.concatenate(cols, axis=1))  # [n_in, 512]
    return out


def _prep_b(bsum):
    out = []
    for hf in range(2):
        cols = [bsum[256 * gi + 128 * hf : 256 * gi + 128 * hf + 128] * (2.0 if gi == 2 else 1.0) for gi in _GORDER]
        out.append(np.concatenate(cols))
    return out


def _install_trace_shim():
    # antenv.axon_hooks is absent in this image; synthesize it from the
    # ctypes hook that ships in trn_agent_boot so trace=True works.
    import sys, types
    try:
        import antenv.axon_hooks  # noqa: F401
        return
    except ImportError:
        pass
    from trn_agent_boot.trn_boot import _ntff_profile_via_ctypes
    hook = _ntff_profile_via_ctypes('/opt/axon/libaxon_pjrt.so')
    mod = types.ModuleType('antenv.axon_hooks')
    mod.get_axon_ntff_profile_hook = lambda: hook
    mod.set_axon_ntff_profile_hook = lambda h: None
    sys.modules['antenv.axon_hooks'] = mod
    import concourse.bass_utils as bu
    bu.upload_artifacts = lambda tmpdir: ""  # zero-egress container


def _run_device(embT_np, wrec_np, wx_np, bias_np, wout_np, trace=False):
    global LAST_EXEC_NS
    from concourse.bass_utils import run_bass_kernel_spmd

    if trace:
        _install_trace_shim()

    nc = _build_bass()
    in_map = {
        "embT": embT_np,
        "wrec": wrec_np,
        "wx": wx_np,
        "biasd": bias_np,
        "woutd": wout_np,
    }
    res = run_bass_kernel_spmd(
        nc, [in_map] * NCORES, core_ids=list(range(NCORES)), trace=trace
    )
    LAST_EXEC_NS = res.exec_time_ns
    global LAST_RESULT
    LAST_RESULT = res
    return res.results[0]["featsT"]


def _host_feats(emb, Wih_f, Whh_f, bih_f, bhh_f, Wih_b, Whh_b, bih_b, bhh_b, Wout, bout):
    xs = np.swapaxes(emb, 0, 1).astype(np.float32)  # [S, B, E]

    def sigmoid(x):
        return 1.0 / (1.0 + np.exp(-x))

    def run_dir(Wih, Whh, bih, bhh, reverse):
        xp = xs @ Wih.T.astype(np.float32) + (bih + bhh).astype(np.float32)
        WhhT = np.ascontiguousarray(Whh.T.astype(np.float32))
        h = np.zeros((B, H), np.float32)
        c = np.zeros((B, H), np.float32)
        hs = np.empty((S, B, H), np.float32)
        order = range(S - 1, -1, -1) if reverse else range(S)
        for s in order:
            gg = xp[s] + h @ WhhT
            i = sigmoid(gg[:, :H])
            f = sigmoid(gg[:, H : 2 * H])
            gq = np.tanh(gg[:, 2 * H : 3 * H])
            o = sigmoid(gg[:, 3 * H :])
            c = f * c + i * gq
            h = o * np.tanh(c)
            hs[s] = h
        return hs

    hf = run_dir(Wih_f, Whh_f, bih_f, bhh_f, False)
    hb = run_dir(Wih_b, Whh_b, bih_b, bhh_b, True)
    hs = np.concatenate([hf, hb], axis=-1)
    return hs @ Wout.T.astype(np.float32) + bout.astype(np.float32)


def kernel(
    sentence,
    embed,
    Wih_f,
    Whh_f,
    bih_f,
    bhh_f,
    Wih_b,
    Whh_b,
    bih_b,
    bhh_b,
    Wout,
    bout,
    transitions,
    start_t,
    stop_t,
):
    import ml_dtypes

    bf16 = ml_dtypes.bfloat16
    sentence = np.asarray(sentence)

    emb = embed.astype(np.float32)[sentence]  # [B, S, E]
    embT_np = np.ascontiguousarray(emb.transpose(2, 1, 0).reshape(E, SBW)).astype(bf16)

    wrec_np = np.concatenate(
        _prep_w(Whh_f.astype(np.float32), H) + _prep_w(Whh_b.astype(np.float32), H),
        axis=1,
    ).astype(bf16)  # [256, 2048]
    wx_np = np.concatenate(
        _prep_w(Wih_f.astype(np.float32), E) + _prep_w(Wih_b.astype(np.float32), E),
        axis=1,
    ).astype(bf16)
    bias_np = np.concatenate(
        _prep_b((bih_f + bhh_f).astype(np.float32))
        + _prep_b((bih_b + bhh_b).astype(np.float32))
    ).reshape(1, 2048).astype(bf16)
    wo = Wout.astype(np.float32)  # [T, 512]
    wout_np = np.concatenate(
        [wo[:, 256 * d + 128 * hf : 256 * d + 128 * hf + 128].T for d in range(2) for hf in range(2)],
        axis=1,
    ).astype(bf16)  # [128, 96]

    trace = bool(int(os.environ.get("KERNEL_TRACE", "0")))
    try:
        featsT = _run_device(embT_np, wrec_np, wx_np, bias_np, wout_np, trace=trace)
        ff = np.asarray(featsT[:, :SBW], dtype=np.float32).reshape(T, S, B)
        fb = np.asarray(featsT[:, SBW:], dtype=np.float32).reshape(T, S, B)[:, ::-1, :]
        feats = (ff + fb).transpose(1, 2, 0)  # [S, B, T]
        feats = feats + bout.astype(np.float32)
    except Exception:
        import traceback

        traceback.print_exc()
        feats = _host_feats(
            emb, Wih_f, Whh_f, bih_f, bhh_f, Wih_b, Whh_b, bih_b, bhh_b, Wout, bout
        )

    # ---- Viterbi (host) ----
    trans = transitions.astype(np.float32)
    v = feats[0] + start_t.astype(np.float32)[None, :]  # [B, T]
    idxs = np.empty((S - 1, B, T), dtype=np.int32)
    for s in range(1, S):
        scores = v[:, :, None] + trans[None]  # [B, prevT, nextT]
        idxs[s - 1] = np.argmax(scores, axis=1)
        v = np.max(scores, axis=1) + feats[s]
    last = np.argmax(v + stop_t.astype(np.float32)[None, :], axis=1).astype(np.int32)

    tags = np.empty((S, B), dtype=np.int32)
    tags[S - 1] = last
    cur = last
    ar = np.arange(B)
    for s in range(S - 2, -1, -1):
        cur = idxs[s][ar, cur].astype(np.int32)
        tags[s] = cur
    return np.ascontiguousarray(tags.T).astype(np.int32)  # [B, S]
